# revision 20
# baseline (speedup 1.0000x reference)
"""Causal multi-head attention (B=2, T=2048, E=1024, 16 heads) on 8 TRN2 cores.

Sharding: 8-way tensor-parallel over heads (2 heads/core) for QKV projections
and attention; one AllToAll per head-half re-shards the attention output over
tokens so each core computes the output projection for its 512-token block.

Final design (v9):
- head-staggered pipeline: h1 attention lags h0 by 2 chunks so QKV, h0
  and h1 phases interleave on PE/ACT/DVE.
- eye DMA first + 36 warmup matmuls on it: HAM flips to K=8/8 during the
  initial DMA wait (framework preamble is ~11us; warmups dovetail into
  the chunk-0 data arrival). First real matmul at ~10us (was 18.4us).
- xP host layout [p, t, e, n]: every chunk is ONE DMA with 8KB
  contiguous per partition (was 1KB packets + 8 DGE ops for chunk 0).
- ONE 1MB AllToAll for both head-halves (collectives are latency/skew
  dominated: 512B dummy ~ 512KB real), then a single merged output
  projection accumulating all 8 source-core feature tiles per PSUM pass
  (no yacc accumulator round-trip, no wo pair-interleave).
- dummy warm A2A late (t==NCH-1): warms the CC stream AND re-syncs the
  cores so thermal-throttle skew is absorbed while ~60us of attention
  work remains. Placed late because collective_compute BLOCKS the GpSimd
  queue until the collective completes, and the CC stream init op can
  run until ~70us — an earlier dummy queues behind it and stalls the
  normalize chain (25-38us PE gaps observed with dummies at t=4/5/6).
- y stored bf16 (halves the final DMA; host casts back to f32).

Measured floor notes: PE busy ~162us at the chip's sustained 13/16
clock; score row-packing via tile_position (64-row pairs) measured ZERO
concurrency on this hardware (start deltas ~= serial), fp8 fails the
2e-2 gate (~5% error for any fp8 dot operand), and the AV ones-row
(M=65) is cycle-neutral since matmul cost is moving-column-bound.
"""
import sys

if "/opt/trn_rl_repo" not in sys.path:
    sys.path.insert(0, "/opt/trn_rl_repo")

import numpy as np

import concourse.bacc as bacc
import concourse.mybir as mybir
from concourse import tile
from concourse.bass_utils import run_bass_kernel_spmd

dt = mybir.dt
AF = mybir.ActivationFunctionType
ALU = mybir.AluOpType

B, T, E, HS, NH = 2, 2048, 1024, 64, 16
NCORE = 8
NTOK = B * T            # 4096
CH = 512                # token chunk
NCH = NTOK // CH        # 8
CPB = NCH // B          # chunks per batch = 4
SUB = 128
NSUB = CH // SUB        # 4

_nc_cache = {}


def build_nc():
    nc = bacc.Bacc("TRN2", target_bir_lowering=False, debug=False,
                   num_devices=NCORE)
    f32, bf16 = dt.float32, dt.bfloat16

    xP = nc.declare_dram_parameter("xP", [128, NCH, 8, CH], bf16,
                                   isOutput=False)
    wqT = nc.declare_dram_parameter("wqT", [128, 8, 128], bf16,
                                    isOutput=False)
    wkT = nc.declare_dram_parameter("wkT", [128, 8, 128], bf16,
                                    isOutput=False)
    wvT = nc.declare_dram_parameter("wvT", [128, 8, 128], bf16,
                                    isOutput=False)
    woAll = nc.declare_dram_parameter("woAll", [128, 8, E], bf16,
                                      isOutput=False)
    bqs = nc.declare_dram_parameter("bqs", [128, 1], f32, isOutput=False)
    bks = nc.declare_dram_parameter("bks", [128, 1], f32, isOutput=False)
    bvs = nc.declare_dram_parameter("bvs", [128, 1], f32, isOutput=False)
    bo_b = nc.declare_dram_parameter("bo_b", [128, E], f32, isOutput=False)
    eye = nc.declare_dram_parameter("eye", [128, 128], bf16, isOutput=False)
    tri01 = nc.declare_dram_parameter("tri01", [128, 128], bf16,
                                      isOutput=False)
    ones_v = nc.declare_dram_parameter("ones_v", [128, NCH * NSUB], bf16,
                                       isOutput=False)
    y = nc.declare_dram_parameter("y", [CH, E], bf16, isOutput=True)

    with tile.TileContext(nc) as tc:
        from contextlib import ExitStack
        with ExitStack() as top:
            const = top.enter_context(tc.tile_pool(name="const", bufs=1))
            persist = top.enter_context(tc.tile_pool(name="persist", bufs=1))
            xtp_pool = top.enter_context(tc.tile_pool(name="xtp", bufs=2))
            vstage = top.enter_context(tc.tile_pool(name="vstage", bufs=2))
            ppool = top.enter_context(tc.tile_pool(name="ppool", bufs=4))
            apool = top.enter_context(tc.tile_pool(name="apool", bufs=2))
            bcpool = top.enter_context(tc.tile_pool(name="bcpool", bufs=2))
            recpool = top.enter_context(tc.tile_pool(name="recpool", bufs=2))
            ystage = top.enter_context(tc.tile_pool(name="ystage", bufs=2))
            ps_q = top.enter_context(
                tc.tile_pool(name="ps_q", bufs=2, space="PSUM"))
            ps_s = top.enter_context(
                tc.tile_pool(name="ps_s", bufs=2, space="PSUM"))
            ps_a = top.enter_context(
                tc.tile_pool(name="ps_a", bufs=2, space="PSUM"))
            dram = top.enter_context(
                tc.tile_pool(name="dram", bufs=1, space="DRAM"))

            # ---- eye first: unblocks the HAM warmup matmuls ---------------
            eye_sb = const.tile([128, 128], bf16, name="eye_sb")
            nc.sync.dma_start(eye_sb[:], eye[:])

            # ---- HAM warmup: PE busy during the initial DMA wait ----------
            wps = ps_q.tile([128, 128], f32, name="wps", tag="psq")
            for _ in range(46):
                nc.tensor.matmul(wps[:], eye_sb[:], eye_sb[:],
                                 start=True, stop=True)

            # ---- persistent weights + chunk-0 data ------------------------
            wq_sb = persist.tile([128, 8, 128], bf16, name="wq_sb")
            wk_sb = persist.tile([128, 8, 128], bf16, name="wk_sb")
            wv_sb = persist.tile([128, 8, 128], bf16, name="wv_sb")
            nc.sync.dma_start(wq_sb[:], wqT[:])
            xTt0 = xtp_pool.tile([128, 8, CH], bf16, name="xTt", tag="xTt")
            nc.sync.dma_start(xTt0[:], xP[:, 0])
            nc.sync.dma_start(wk_sb[:], wkT[:])
            nc.sync.dma_start(wv_sb[:], wvT[:])

            # ---- remaining constants --------------------------------------
            onesv_sb = const.tile([128, NCH * NSUB], bf16, name="onesv_sb")
            nc.sync.dma_start(onesv_sb[:], ones_v[:])
            bq_sb = const.tile([128, 1], f32, name="bq_sb")
            nc.sync.dma_start(bq_sb[:], bqs[:])
            bk_sb = const.tile([128, 1], f32, name="bk_sb")
            nc.sync.dma_start(bk_sb[:], bks[:])
            bv_sb = const.tile([128, 1], f32, name="bv_sb")
            nc.sync.dma_start(bv_sb[:], bvs[:])
            tri_sb = const.tile([128, 128], bf16, name="tri_sb")
            nc.sync.dma_start(tri_sb[:], tri01[:])
            bo_sb = const.tile([128, E], f32, name="bo_sb")
            nc.sync.dma_start(bo_sb[:], bo_b[:])

            # ---- persistent activations -----------------------------------
            kT = persist.tile([128, NCH, CH], bf16, name="kT")
            qT = persist.tile([128, NCH, CH], bf16, name="qT")
            # V rows per k-token group g; cols 0:64 = h0 feats, 64 = ones,
            # 65:129 = h1 feats, 129 = ones.  AV stationary h = [:, g,
            # 65h:65h+65]; the ones row makes the AV matmul emit softmax
            # denominators in PSUM row 64.
            vh = persist.tile([128, NCH * NSUB, 130], bf16, name="vh")
            nc.vector.tensor_copy(vh[:, :, 64], onesv_sb[:])
            nc.vector.tensor_copy(vh[:, :, 129], onesv_sb[:])

            wo_sb = persist.tile([128, 8, E], bf16, name="wo_sb")

            # single A2A buffer: both head-halves stacked on the feat dim.
            # core c sends chunk j (its 128 feats x 512 tokens) to core j.
            cc_in = dram.tile([NCH, 128, CH], bf16, name="cc_in")
            cc_out = dram.tile([NCH, 128, CH], bf16, name="cc_out")
            warm_in = dram.tile([NCH, 1, 32], bf16, name="warm_in")
            warm_out = dram.tile([NCH, 1, 32], bf16, name="warm_out")
            nc.sync.dma_start(warm_in[:, 0, :], onesv_sb[0:8, 0:32])

            # ---- phase B: QKV projection for one token chunk ---------------
            def emit_b(t):
                if t == 0:
                    xTt = xTt0
                else:
                    xTt = xtp_pool.tile([128, 8, CH], bf16, name="xTt",
                                        tag="xTt")
                    nc.sync.dma_start(xTt[:], xP[:, t])
                for wsb, bias, scale, dest in (
                        (wq_sb, bq_sb, 0.125, qT),
                        (wk_sb, bk_sb, None, kT)):
                    ps = ps_q.tile([128, CH], f32, name="psqk", tag="psq")
                    for e in range(8):
                        nc.tensor.matmul(ps[:], wsb[:, e, :], xTt[:, e, :],
                                         start=(e == 0), stop=(e == 7))
                    if scale is None:
                        nc.vector.tensor_scalar_add(dest[:, t, :], ps[:],
                                                    bias[:])
                    else:
                        nc.vector.tensor_scalar(
                            dest[:, t, :], ps[:], scale, bias[:],
                            ALU.mult, ALU.add)

                psv = ps_q.tile([128, CH], f32, name="psv", tag="psq")
                for e in range(8):
                    nc.tensor.matmul(psv[:], wv_sb[:, e, :], xTt[:, e, :],
                                     start=(e == 0), stop=(e == 7))
                vTs = vstage.tile([128, CH], bf16, name="vTs", tag="vTs")
                nc.vector.tensor_scalar_add(vTs[:], psv[:], bv_sb[:])
                for s in range(NSUB):
                    tv = ps_q.tile([128, 128], bf16, name="tv", tag="psq")
                    nc.tensor.transpose(
                        tv[:], vTs[:, 128 * s:128 * (s + 1)], eye_sb[:])
                    g = NSUB * t + s
                    nc.vector.tensor_copy(vh[:, g, 0:64], tv[:, 0:64])
                    nc.vector.tensor_copy(vh[:, g, 65:129], tv[:, 64:128])

            # ---- phase C: attention for one (chunk, head-half) -------------
            def emit_c(t, h):
                b0 = CPB * (t // CPB)
                pb = 64 * h
                a_ps = ps_a.tile([128, CH], f32, name="a_ps", tag="aps")

                def emit_scores(kc):
                    diag = kc == t
                    pT = ppool.tile([128, NSUB, CH], bf16, name="pT",
                                    tag="pT")
                    for j in range(2):
                        sp = ps_s.tile([128, 2 * CH], f32, name="sp",
                                       tag="sps")
                        for jj in range(2):
                            s = 2 * j + jj
                            q0 = SUB * s if diag else 0
                            nc.tensor.matmul(
                                sp[:, CH * jj + q0:CH * jj + CH],
                                kT[pb:pb + 64, kc, SUB * s:SUB * (s + 1)],
                                qT[pb:pb + 64, t, q0:CH],
                                start=True, stop=True)
                        if diag:
                            for jj in range(2):
                                s = 2 * j + jj
                                q0 = SUB * s
                                nc.scalar.activation(
                                    pT[:, s, q0:CH],
                                    sp[:, CH * jj + q0:CH * jj + CH], AF.Exp)
                                nc.vector.tensor_mul(
                                    pT[:, s, q0:q0 + SUB],
                                    pT[:, s, q0:q0 + SUB], tri_sb[:])
                        else:
                            nc.scalar.activation(
                                pT[:, 2 * j:2 * j + 2, :], sp[:], AF.Exp)
                    return pT

                def emit_av(kc, pT):
                    diag = kc == t
                    for s in range(NSUB):
                        q0 = SUB * s if diag else 0
                        g = NSUB * kc + s
                        nc.tensor.matmul(
                            a_ps[0:65, q0:CH], vh[:, g, 65 * h:65 * h + 65],
                            pT[:, s, q0:CH],
                            start=(kc == b0 and s == 0),
                            stop=(diag and s == NSUB - 1))

                prev = None
                for kc in range(b0, t + 1):
                    pT = emit_scores(kc)
                    if prev is not None:
                        emit_av(*prev)
                    prev = (kc, pT)
                emit_av(*prev)

                den = recpool.tile([1, CH], f32, name="den", tag="den")
                nc.vector.tensor_copy(den[:], a_ps[64:65, :])
                rec = recpool.tile([1, CH], f32, name="rec", tag="rec")
                nc.vector.reciprocal_approx_fast(out=rec[:], in_=den[:])
                bc = bcpool.tile([64, CH], f32, name="bc", tag="bc")
                nc.gpsimd.partition_broadcast(bc[:], rec[:])
                a_sb = apool.tile([64, CH], bf16, name="a_sb", tag="asb")
                nc.vector.tensor_mul(a_sb[:], a_ps[0:64, :], bc[:])
                nc.sync.dma_start(cc_in[t, 64 * h:64 * h + 64, :], a_sb[:])

            # ---- main pipeline: QKV(t) | h0-attn(t-1) | h1-attn(t-2) ------
            for t in range(NCH):
                emit_b(t)
                if t == NCH - 1:
                    # tiny dummy AllToAll. Three roles: (1) keeps the CC
                    # stream warm for the real A2A; (2) re-syncs the cores
                    # here so accumulated thermal-throttle skew is absorbed
                    # while ~60us of attention work remains; (3) placed THIS
                    # late because collective_compute blocks the GpSimd
                    # queue until the collective completes, and the CC
                    # stream's init op can run until ~70us — an earlier
                    # dummy can queue behind it and stall the normalize
                    # chain (25-38us PE gaps observed at t==4/5/6).
                    nc.gpsimd.collective_compute(
                        "AllToAll", ALU.bypass,
                        ins=[warm_in.opt()], outs=[warm_out.opt()],
                        replica_groups=[list(range(NCORE))])
                if t >= 1:
                    emit_c(t - 1, 0)
                if t >= 2:
                    emit_c(t - 2, 1)
            # wo weights: DMA-idle window once all xP chunks are in flight
            nc.sync.dma_start(wo_sb[:], woAll[:])

            emit_c(NCH - 1, 0)
            emit_c(NCH - 2, 1)
            emit_c(NCH - 1, 1)
            # single A2A: both heads, all chunks (collectives here are
            # latency/skew-dominated, so one 1MB A2A beats two serialized
            # 512KB ones on the single CC stream)
            nc.gpsimd.collective_compute(
                "AllToAll", ALU.bypass,
                ins=[cc_in.opt()], outs=[cc_out.opt()],
                replica_groups=[list(range(NCORE))])

            # keep-warm: the A2A wait is 10-90us of PE idle; without this
            # the HAM re-throttles to K=4/8 and the whole output projection
            # runs at half clock (~9us loss, 634ns/mm observed). ~11us of
            # tiny matmuls keeps the array busy through the typical wait;
            # they overlap the collective and only delay the projection
            # when the A2A returns faster than that.
            wps2 = ps_q.tile([128, 64], f32, name="wps2", tag="psq")
            for _ in range(180):
                nc.tensor.matmul(wps2[:], eye_sb[:], eye_sb[:, 0:64],
                                 start=True, stop=True)

            # aTb[f, j, n]: feats of source core j for our 512 tokens.
            # Two tiles so the first projection chain (j=0..3) starts after
            # half the re-shard fetch instead of the full 1MB.
            aTbA = xtp_pool.tile([128, 4, CH], bf16, name="aTbA", tag="aTbA")
            aTbB = xtp_pool.tile([128, 4, CH], bf16, name="aTbB", tag="aTbB")
            nc.sync.dma_start(
                aTbA[:], cc_out[0:4].rearrange("j f n -> f j n"))
            nc.sync.dma_start(
                aTbB[:], cc_out[4:8].rearrange("j f n -> f j n"))

            # ---- output projection + store --------------------------------
            for m in range(NSUB):
                for nchk in range(2):
                    yps = ps_q.tile([128, CH], f32, name="yps", tag="psq")
                    for j in range(8):
                        src = aTbA if j < 4 else aTbB
                        nc.tensor.matmul(
                            yps[:],
                            src[:, j % 4, SUB * m:SUB * (m + 1)],
                            wo_sb[:, j, CH * nchk:CH * (nchk + 1)],
                            start=(j == 0), stop=(j == 7))
                    ysb = ystage.tile([128, CH], bf16, name="ysb", tag="ysb")
                    nc.vector.tensor_add(
                        ysb[:], yps[:],
                        bo_sb[:, CH * nchk:CH * (nchk + 1)])
                    nc.sync.dma_start(
                        y[SUB * m:SUB * (m + 1),
                          CH * nchk:CH * (nchk + 1)],
                        ysb[:])
    nc.compile()
    return nc


def _prep_in_maps(embd_q, Wq, bq, Wk, bk, Wv, bv, Wo, bo):
    import ml_dtypes
    bf16 = ml_dtypes.bfloat16
    x = embd_q.reshape(NTOK, E).astype(np.float32)
    # xP[p, t, e, n] = x[t*512+n, e*128+p]: 8KB contiguous per partition
    # per chunk
    xPm = np.ascontiguousarray(
        x.reshape(NCH, CH, 8, 128).transpose(3, 0, 2, 1).astype(bf16))
    eye = np.eye(128, dtype=bf16)
    r = np.arange(128)
    # pT is [k-part, q-col]; mask out k > q (future tokens)
    tri01 = np.ascontiguousarray(
        np.where(r[:, None] > r[None, :], 0.0, 1.0).astype(bf16))
    ones_v = np.ones((128, NCH * NSUB), dtype=bf16)
    bo_b = np.ascontiguousarray(
        np.broadcast_to(bo.astype(np.float32), (128, E)))
    woTf = Wo.astype(np.float32).T  # [feat, out]
    # woAll[p, j, o] = Wo.T[128j + p, o]: cc_out[j] row p is global feat
    # 128j + p, so the projection contracts per-source-core tiles
    woAllm = np.ascontiguousarray(
        woTf.reshape(8, 128, E).transpose(1, 0, 2).astype(bf16))

    def wlayout(W, sl):
        # [E, 128] -> [p, e, m]: contiguous 2KB/partition DMA segments
        wT = W[sl].astype(np.float32).T.astype(bf16)
        return np.ascontiguousarray(
            wT.reshape(8, 128, 128).transpose(1, 0, 2))

    in_maps = []
    for c in range(NCORE):
        sl = slice(128 * c, 128 * (c + 1))
        in_maps.append({
            "xP": xPm,
            "wqT": wlayout(Wq, sl),
            "wkT": wlayout(Wk, sl),
            "wvT": wlayout(Wv, sl),
            "woAll": woAllm,
            "bqs": np.ascontiguousarray(
                (bq[sl] * 0.125).reshape(128, 1), dtype=np.float32),
            "bks": np.ascontiguousarray(bk[sl].reshape(128, 1),
                                        dtype=np.float32),
            "bvs": np.ascontiguousarray(bv[sl].reshape(128, 1),
                                        dtype=np.float32),
            "bo_b": bo_b,
            "eye": eye,
            "tri01": tri01,
            "ones_v": ones_v,
        })
    return in_maps


def kernel(embd_q, Wq, bq, Wk, bk, Wv, bv, Wo, bo, _trace=False):
    if "nc" not in _nc_cache:
        _nc_cache["nc"] = build_nc()
    in_maps = _prep_in_maps(np.asarray(embd_q), np.asarray(Wq), np.asarray(bq),
                            np.asarray(Wk), np.asarray(bk), np.asarray(Wv),
                            np.asarray(bv), np.asarray(Wo), np.asarray(bo))
    import os
    tc_env = os.environ.get("TRACE_CORES")
    res = run_bass_kernel_spmd(
        _nc_cache["nc"], in_maps, list(range(NCORE)), trace=_trace,
        trace_cores=(list(range(NCORE)) if tc_env else None))
    out = np.concatenate(
        [np.asarray(res.results[c]["y"]).astype(np.float32)
         for c in range(NCORE)], axis=0)
    out = out.reshape(B, T, E)
    kernel.last_results = res
    return out


# revision 22
# speedup vs baseline: 1.0116x; 1.0116x over previous
"""Causal multi-head attention (B=2, T=2048, E=1024, 16 heads) on 8 TRN2 cores.

Sharding: 8-way tensor-parallel over heads (2 heads/core) for QKV projections
and attention; one AllToAll per head-half re-shards the attention output over
tokens so each core computes the output projection for its 512-token block.

Final design (v9):
- head-staggered pipeline: h1 attention lags h0 by 2 chunks so QKV, h0
  and h1 phases interleave on PE/ACT/DVE.
- eye DMA first + 36 warmup matmuls on it: HAM flips to K=8/8 during the
  initial DMA wait (framework preamble is ~11us; warmups dovetail into
  the chunk-0 data arrival). First real matmul at ~10us (was 18.4us).
- xP host layout [p, t, e, n]: every chunk is ONE DMA with 8KB
  contiguous per partition (was 1KB packets + 8 DGE ops for chunk 0).
- ONE 1MB AllToAll for both head-halves (collectives are latency/skew
  dominated: 512B dummy ~ 512KB real), then a single merged output
  projection accumulating all 8 source-core feature tiles per PSUM pass
  (no yacc accumulator round-trip, no wo pair-interleave).
- dummy warm A2A late (t==NCH-1): warms the CC stream AND re-syncs the
  cores so thermal-throttle skew is absorbed while ~60us of attention
  work remains. Placed late because collective_compute BLOCKS the GpSimd
  queue until the collective completes, and the CC stream init op can
  run until ~70us — an earlier dummy queues behind it and stalls the
  normalize chain (25-38us PE gaps observed with dummies at t=4/5/6).
- y stored bf16 (halves the final DMA; host casts back to f32).

Measured floor notes: PE busy ~162us at the chip's sustained 13/16
clock; score row-packing via tile_position (64-row pairs) measured ZERO
concurrency on this hardware (start deltas ~= serial), fp8 fails the
2e-2 gate (~5% error for any fp8 dot operand), and the AV ones-row
(M=65) is cycle-neutral since matmul cost is moving-column-bound.
"""
import sys

if "/opt/trn_rl_repo" not in sys.path:
    sys.path.insert(0, "/opt/trn_rl_repo")

import numpy as np

import concourse.bacc as bacc
import concourse.mybir as mybir
from concourse import tile
from concourse.bass_utils import run_bass_kernel_spmd

dt = mybir.dt
AF = mybir.ActivationFunctionType
ALU = mybir.AluOpType

B, T, E, HS, NH = 2, 2048, 1024, 64, 16
NCORE = 8
NTOK = B * T            # 4096
CH = 512                # token chunk
NCH = NTOK // CH        # 8
CPB = NCH // B          # chunks per batch = 4
SUB = 128
NSUB = CH // SUB        # 4

_nc_cache = {}


def build_nc():
    nc = bacc.Bacc("TRN2", target_bir_lowering=False, debug=False,
                   num_devices=NCORE)
    f32, bf16 = dt.float32, dt.bfloat16

    xP = nc.declare_dram_parameter("xP", [128, NCH, 8, CH], bf16,
                                   isOutput=False)
    wqT = nc.declare_dram_parameter("wqT", [128, 8, 128], bf16,
                                    isOutput=False)
    wkT = nc.declare_dram_parameter("wkT", [128, 8, 128], bf16,
                                    isOutput=False)
    wvT = nc.declare_dram_parameter("wvT", [128, 8, 128], bf16,
                                    isOutput=False)
    woAll = nc.declare_dram_parameter("woAll", [128, 8, E], bf16,
                                      isOutput=False)
    bqs = nc.declare_dram_parameter("bqs", [128, 1], f32, isOutput=False)
    bks = nc.declare_dram_parameter("bks", [128, 1], f32, isOutput=False)
    bvs = nc.declare_dram_parameter("bvs", [128, 1], f32, isOutput=False)
    bo_b = nc.declare_dram_parameter("bo_b", [128, E], f32, isOutput=False)
    eye = nc.declare_dram_parameter("eye", [128, 128], bf16, isOutput=False)
    tri01 = nc.declare_dram_parameter("tri01", [128, 128], bf16,
                                      isOutput=False)
    ones_v = nc.declare_dram_parameter("ones_v", [128, NCH * NSUB], bf16,
                                       isOutput=False)
    y = nc.declare_dram_parameter("y", [CH, E], bf16, isOutput=True)

    with tile.TileContext(nc) as tc:
        from contextlib import ExitStack
        with ExitStack() as top:
            const = top.enter_context(tc.tile_pool(name="const", bufs=1))
            persist = top.enter_context(tc.tile_pool(name="persist", bufs=1))
            xtp_pool = top.enter_context(tc.tile_pool(name="xtp", bufs=2))
            vstage = top.enter_context(tc.tile_pool(name="vstage", bufs=2))
            ppool = top.enter_context(tc.tile_pool(name="ppool", bufs=4))
            apool = top.enter_context(tc.tile_pool(name="apool", bufs=2))
            bcpool = top.enter_context(tc.tile_pool(name="bcpool", bufs=2))
            recpool = top.enter_context(tc.tile_pool(name="recpool", bufs=2))
            ystage = top.enter_context(tc.tile_pool(name="ystage", bufs=2))
            ps_q = top.enter_context(
                tc.tile_pool(name="ps_q", bufs=2, space="PSUM"))
            ps_s = top.enter_context(
                tc.tile_pool(name="ps_s", bufs=2, space="PSUM"))
            ps_a = top.enter_context(
                tc.tile_pool(name="ps_a", bufs=2, space="PSUM"))
            dram = top.enter_context(
                tc.tile_pool(name="dram", bufs=1, space="DRAM"))

            # ---- eye first: unblocks the HAM warmup matmuls ---------------
            eye_sb = const.tile([128, 128], bf16, name="eye_sb")
            nc.sync.dma_start(eye_sb[:], eye[:])

            # ---- HAM warmup: PE busy during the initial DMA wait ----------
            wps = ps_q.tile([128, 128], f32, name="wps", tag="psq")
            for _ in range(46):
                nc.tensor.matmul(wps[:], eye_sb[:], eye_sb[:],
                                 start=True, stop=True)

            # ---- persistent weights + chunk-0 data ------------------------
            wq_sb = persist.tile([128, 8, 128], bf16, name="wq_sb")
            wk_sb = persist.tile([128, 8, 128], bf16, name="wk_sb")
            wv_sb = persist.tile([128, 8, 128], bf16, name="wv_sb")
            nc.sync.dma_start(wq_sb[:], wqT[:])
            xTt0 = xtp_pool.tile([128, 8, CH], bf16, name="xTt", tag="xTt")
            nc.sync.dma_start(xTt0[:], xP[:, 0])
            nc.sync.dma_start(wk_sb[:], wkT[:])
            nc.sync.dma_start(wv_sb[:], wvT[:])

            # ---- remaining constants --------------------------------------
            onesv_sb = const.tile([128, NCH * NSUB], bf16, name="onesv_sb")
            nc.sync.dma_start(onesv_sb[:], ones_v[:])
            bq_sb = const.tile([128, 1], f32, name="bq_sb")
            nc.sync.dma_start(bq_sb[:], bqs[:])
            bk_sb = const.tile([128, 1], f32, name="bk_sb")
            nc.sync.dma_start(bk_sb[:], bks[:])
            bv_sb = const.tile([128, 1], f32, name="bv_sb")
            nc.sync.dma_start(bv_sb[:], bvs[:])
            tri_sb = const.tile([128, 128], bf16, name="tri_sb")
            nc.sync.dma_start(tri_sb[:], tri01[:])
            bo_sb = const.tile([128, E], f32, name="bo_sb")
            nc.sync.dma_start(bo_sb[:], bo_b[:])

            # ---- persistent activations -----------------------------------
            kT = persist.tile([128, NCH, CH], bf16, name="kT")
            qT = persist.tile([128, NCH, CH], bf16, name="qT")
            # V rows per k-token group g; cols 0:64 = h0 feats, 64 = ones,
            # 65:129 = h1 feats, 129 = ones.  AV stationary h = [:, g,
            # 65h:65h+65]; the ones row makes the AV matmul emit softmax
            # denominators in PSUM row 64.
            vh = persist.tile([128, NCH * NSUB, 130], bf16, name="vh")
            nc.vector.tensor_copy(vh[:, :, 64], onesv_sb[:])
            nc.vector.tensor_copy(vh[:, :, 129], onesv_sb[:])

            wo_sb = persist.tile([128, 8, E], bf16, name="wo_sb")

            # single A2A buffer: both head-halves stacked on the feat dim.
            # core c sends chunk j (its 128 feats x 512 tokens) to core j.
            cc_in = dram.tile([NCH, 128, CH], bf16, name="cc_in")
            cc_out = dram.tile([NCH, 128, CH], bf16, name="cc_out")
            warm_in = dram.tile([NCH, 1, 32], bf16, name="warm_in")
            warm_out = dram.tile([NCH, 1, 32], bf16, name="warm_out")
            nc.sync.dma_start(warm_in[:, 0, :], onesv_sb[0:8, 0:32])

            # ---- phase B: QKV projection for one token chunk ---------------
            def emit_b(t):
                if t == 0:
                    xTt = xTt0
                else:
                    xTt = xtp_pool.tile([128, 8, CH], bf16, name="xTt",
                                        tag="xTt")
                    nc.sync.dma_start(xTt[:], xP[:, t])
                for wsb, bias, scale, dest in (
                        (wq_sb, bq_sb, 0.125, qT),
                        (wk_sb, bk_sb, None, kT)):
                    ps = ps_q.tile([128, CH], f32, name="psqk", tag="psq")
                    for e in range(8):
                        nc.tensor.matmul(ps[:], wsb[:, e, :], xTt[:, e, :],
                                         start=(e == 0), stop=(e == 7))
                    if scale is None:
                        nc.vector.tensor_scalar_add(dest[:, t, :], ps[:],
                                                    bias[:])
                    else:
                        nc.vector.tensor_scalar(
                            dest[:, t, :], ps[:], scale, bias[:],
                            ALU.mult, ALU.add)

                psv = ps_q.tile([128, CH], f32, name="psv", tag="psq")
                for e in range(8):
                    nc.tensor.matmul(psv[:], wv_sb[:, e, :], xTt[:, e, :],
                                     start=(e == 0), stop=(e == 7))
                vTs = vstage.tile([128, CH], bf16, name="vTs", tag="vTs")
                nc.vector.tensor_scalar_add(vTs[:], psv[:], bv_sb[:])
                for s in range(NSUB):
                    tv = ps_q.tile([128, 128], bf16, name="tv", tag="psq")
                    nc.tensor.transpose(
                        tv[:], vTs[:, 128 * s:128 * (s + 1)], eye_sb[:])
                    g = NSUB * t + s
                    nc.vector.tensor_copy(vh[:, g, 0:64], tv[:, 0:64])
                    nc.vector.tensor_copy(vh[:, g, 65:129], tv[:, 64:128])

            # ---- phase C: attention for one (chunk, head-half) -------------
            def emit_c(t, h):
                b0 = CPB * (t // CPB)
                pb = 64 * h
                a_ps = ps_a.tile([128, CH], f32, name="a_ps", tag="aps")

                def emit_scores(kc):
                    diag = kc == t
                    pT = ppool.tile([128, NSUB, CH], bf16, name="pT",
                                    tag="pT")
                    for j in range(2):
                        sp = ps_s.tile([128, 2 * CH], f32, name="sp",
                                       tag="sps")
                        for jj in range(2):
                            s = 2 * j + jj
                            q0 = SUB * s if diag else 0
                            nc.tensor.matmul(
                                sp[:, CH * jj + q0:CH * jj + CH],
                                kT[pb:pb + 64, kc, SUB * s:SUB * (s + 1)],
                                qT[pb:pb + 64, t, q0:CH],
                                start=True, stop=True)
                        if diag:
                            for jj in range(2):
                                s = 2 * j + jj
                                q0 = SUB * s
                                nc.scalar.activation(
                                    pT[:, s, q0:CH],
                                    sp[:, CH * jj + q0:CH * jj + CH], AF.Exp)
                                nc.vector.tensor_mul(
                                    pT[:, s, q0:q0 + SUB],
                                    pT[:, s, q0:q0 + SUB], tri_sb[:])
                        else:
                            nc.scalar.activation(
                                pT[:, 2 * j:2 * j + 2, :], sp[:], AF.Exp)
                    return pT

                def emit_av(kc, pT):
                    diag = kc == t
                    for s in range(NSUB):
                        q0 = SUB * s if diag else 0
                        g = NSUB * kc + s
                        nc.tensor.matmul(
                            a_ps[0:65, q0:CH], vh[:, g, 65 * h:65 * h + 65],
                            pT[:, s, q0:CH],
                            start=(kc == b0 and s == 0),
                            stop=(diag and s == NSUB - 1))

                prev = None
                for kc in range(b0, t + 1):
                    pT = emit_scores(kc)
                    if prev is not None:
                        emit_av(*prev)
                    prev = (kc, pT)
                emit_av(*prev)

                # NOTE: the copy is load-bearing — it relocates PSUM row 64
                # to partition 0 (DVE custom ops are lane-aligned and cannot
                # cross partitions; reading a_ps[64:65] directly produces
                # garbage, verified on HW).
                den = recpool.tile([1, CH], f32, name="den", tag="den")
                nc.vector.tensor_copy(den[:], a_ps[64:65, :])
                rec = recpool.tile([1, CH], f32, name="rec", tag="rec")
                nc.vector.reciprocal_approx_fast(out=rec[:], in_=den[:])
                bc = bcpool.tile([64, CH], f32, name="bc", tag="bc")
                nc.gpsimd.partition_broadcast(bc[:], rec[:])
                a_sb = apool.tile([64, CH], bf16, name="a_sb", tag="asb")
                nc.vector.tensor_mul(a_sb[:], a_ps[0:64, :], bc[:])
                nc.sync.dma_start(cc_in[t, 64 * h:64 * h + 64, :], a_sb[:])

            # ---- main pipeline: QKV(t) | h0-attn(t-1) | h1-attn(t-2) ------
            for t in range(NCH):
                emit_b(t)
                if t == NCH - 1:
                    # tiny dummy AllToAll. Three roles: (1) keeps the CC
                    # stream warm for the real A2A; (2) re-syncs the cores
                    # here so accumulated thermal-throttle skew is absorbed
                    # while ~60us of attention work remains; (3) placed THIS
                    # late because collective_compute blocks the GpSimd
                    # queue until the collective completes, and the CC
                    # stream's init op can run until ~70us — an earlier
                    # dummy can queue behind it and stall the normalize
                    # chain (25-38us PE gaps observed at t==4/5/6).
                    nc.gpsimd.collective_compute(
                        "AllToAll", ALU.bypass,
                        ins=[warm_in.opt()], outs=[warm_out.opt()],
                        replica_groups=[list(range(NCORE))])
                if t >= 1:
                    emit_c(t - 1, 0)
                if t >= 2:
                    emit_c(t - 2, 1)
            # wo weights: DMA-idle window once all xP chunks are in flight
            nc.sync.dma_start(wo_sb[:], woAll[:])

            emit_c(NCH - 1, 0)
            emit_c(NCH - 2, 1)
            emit_c(NCH - 1, 1)
            # single A2A: both heads, all chunks (collectives here are
            # latency/skew-dominated, so one 1MB A2A beats two serialized
            # 512KB ones on the single CC stream)
            nc.gpsimd.collective_compute(
                "AllToAll", ALU.bypass,
                ins=[cc_in.opt()], outs=[cc_out.opt()],
                replica_groups=[list(range(NCORE))])

            # keep-warm: the A2A wait is 10-90us of PE idle; without this
            # the HAM re-throttles to K=4/8 and the whole output projection
            # runs at half clock (~9us loss, 634ns/mm observed). ~11us of
            # tiny matmuls keeps the array busy through the typical wait;
            # they overlap the collective and only delay the projection
            # when the A2A returns faster than that.
            wps2 = ps_q.tile([128, 64], f32, name="wps2", tag="psq")
            for _ in range(180):
                nc.tensor.matmul(wps2[:], eye_sb[:], eye_sb[:, 0:64],
                                 start=True, stop=True)

            # aTb[f, j, n]: feats of source core j for our 512 tokens.
            # Two tiles so the first projection chain (j=0..3) starts after
            # half the re-shard fetch instead of the full 1MB.
            aTbA = xtp_pool.tile([128, 4, CH], bf16, name="aTbA", tag="aTbA")
            aTbB = xtp_pool.tile([128, 4, CH], bf16, name="aTbB", tag="aTbB")
            nc.sync.dma_start(
                aTbA[:], cc_out[0:4].rearrange("j f n -> f j n"))
            nc.sync.dma_start(
                aTbB[:], cc_out[4:8].rearrange("j f n -> f j n"))

            # ---- output projection + store --------------------------------
            for m in range(NSUB):
                for nchk in range(2):
                    yps = ps_q.tile([128, CH], f32, name="yps", tag="psq")
                    for j in range(8):
                        src = aTbA if j < 4 else aTbB
                        nc.tensor.matmul(
                            yps[:],
                            src[:, j % 4, SUB * m:SUB * (m + 1)],
                            wo_sb[:, j, CH * nchk:CH * (nchk + 1)],
                            start=(j == 0), stop=(j == 7))
                    ysb = ystage.tile([128, CH], bf16, name="ysb", tag="ysb")
                    nc.vector.tensor_add(
                        ysb[:], yps[:],
                        bo_sb[:, CH * nchk:CH * (nchk + 1)])
                    nc.sync.dma_start(
                        y[SUB * m:SUB * (m + 1),
                          CH * nchk:CH * (nchk + 1)],
                        ysb[:])
    nc.compile()
    return nc


def _prep_in_maps(embd_q, Wq, bq, Wk, bk, Wv, bv, Wo, bo):
    import ml_dtypes
    bf16 = ml_dtypes.bfloat16
    x = embd_q.reshape(NTOK, E).astype(np.float32)
    # xP[p, t, e, n] = x[t*512+n, e*128+p]: 8KB contiguous per partition
    # per chunk
    xPm = np.ascontiguousarray(
        x.reshape(NCH, CH, 8, 128).transpose(3, 0, 2, 1).astype(bf16))
    eye = np.eye(128, dtype=bf16)
    r = np.arange(128)
    # pT is [k-part, q-col]; mask out k > q (future tokens)
    tri01 = np.ascontiguousarray(
        np.where(r[:, None] > r[None, :], 0.0, 1.0).astype(bf16))
    ones_v = np.ones((128, NCH * NSUB), dtype=bf16)
    bo_b = np.ascontiguousarray(
        np.broadcast_to(bo.astype(np.float32), (128, E)))
    woTf = Wo.astype(np.float32).T  # [feat, out]
    # woAll[p, j, o] = Wo.T[128j + p, o]: cc_out[j] row p is global feat
    # 128j + p, so the projection contracts per-source-core tiles
    woAllm = np.ascontiguousarray(
        woTf.reshape(8, 128, E).transpose(1, 0, 2).astype(bf16))

    def wlayout(W, sl):
        # [E, 128] -> [p, e, m]: contiguous 2KB/partition DMA segments
        wT = W[sl].astype(np.float32).T.astype(bf16)
        return np.ascontiguousarray(
            wT.reshape(8, 128, 128).transpose(1, 0, 2))

    in_maps = []
    for c in range(NCORE):
        sl = slice(128 * c, 128 * (c + 1))
        in_maps.append({
            "xP": xPm,
            "wqT": wlayout(Wq, sl),
            "wkT": wlayout(Wk, sl),
            "wvT": wlayout(Wv, sl),
            "woAll": woAllm,
            "bqs": np.ascontiguousarray(
                (bq[sl] * 0.125).reshape(128, 1), dtype=np.float32),
            "bks": np.ascontiguousarray(bk[sl].reshape(128, 1),
                                        dtype=np.float32),
            "bvs": np.ascontiguousarray(bv[sl].reshape(128, 1),
                                        dtype=np.float32),
            "bo_b": bo_b,
            "eye": eye,
            "tri01": tri01,
            "ones_v": ones_v,
        })
    return in_maps


def kernel(embd_q, Wq, bq, Wk, bk, Wv, bv, Wo, bo, _trace=False):
    if "nc" not in _nc_cache:
        _nc_cache["nc"] = build_nc()
    in_maps = _prep_in_maps(np.asarray(embd_q), np.asarray(Wq), np.asarray(bq),
                            np.asarray(Wk), np.asarray(bk), np.asarray(Wv),
                            np.asarray(bv), np.asarray(Wo), np.asarray(bo))
    import os
    tc_env = os.environ.get("TRACE_CORES")
    res = run_bass_kernel_spmd(
        _nc_cache["nc"], in_maps, list(range(NCORE)), trace=_trace,
        trace_cores=(list(range(NCORE)) if tc_env else None))
    out = np.concatenate(
        [np.asarray(res.results[c]["y"]).astype(np.float32)
         for c in range(NCORE)], axis=0)
    out = out.reshape(B, T, E)
    kernel.last_results = res
    return out


# revision 26
# speedup vs baseline: 1.0676x; 1.0553x over previous
"""Causal multi-head attention (B=2, T=2048, E=1024, 16 heads) on 8 TRN2 cores.

Sharding: 8-way tensor-parallel over heads (2 heads/core) for QKV projections
and attention; one AllToAll per head-half re-shards the attention output over
tokens so each core computes the output projection for its 512-token block.

Final design (v9):
- head-staggered pipeline: h1 attention lags h0 by 2 chunks so QKV, h0
  and h1 phases interleave on PE/ACT/DVE.
- eye DMA first + 36 warmup matmuls on it: HAM flips to K=8/8 during the
  initial DMA wait (framework preamble is ~11us; warmups dovetail into
  the chunk-0 data arrival). First real matmul at ~10us (was 18.4us).
- xP host layout [p, t, e, n]: every chunk is ONE DMA with 8KB
  contiguous per partition (was 1KB packets + 8 DGE ops for chunk 0).
- ONE 1MB AllToAll for both head-halves (collectives are latency/skew
  dominated: 512B dummy ~ 512KB real), then a single merged output
  projection accumulating all 8 source-core feature tiles per PSUM pass
  (no yacc accumulator round-trip, no wo pair-interleave).
- dummy warm A2A late (t==NCH-1): warms the CC stream AND re-syncs the
  cores so thermal-throttle skew is absorbed while ~60us of attention
  work remains. Placed late because collective_compute BLOCKS the GpSimd
  queue until the collective completes, and the CC stream init op can
  run until ~70us — an earlier dummy queues behind it and stalls the
  normalize chain (25-38us PE gaps observed with dummies at t=4/5/6).
- y stored bf16 (halves the final DMA; host casts back to f32).

Measured floor notes: PE busy ~162us at the chip's sustained 13/16
clock; score row-packing via tile_position (64-row pairs) measured ZERO
concurrency on this hardware (start deltas ~= serial), fp8 fails the
2e-2 gate (~5% error for any fp8 dot operand), and the AV ones-row
(M=65) is cycle-neutral since matmul cost is moving-column-bound.
"""
import sys

if "/opt/trn_rl_repo" not in sys.path:
    sys.path.insert(0, "/opt/trn_rl_repo")

import numpy as np

import concourse.bacc as bacc
import concourse.mybir as mybir
from concourse import tile
from concourse.bass_utils import run_bass_kernel_spmd

dt = mybir.dt
AF = mybir.ActivationFunctionType
ALU = mybir.AluOpType

B, T, E, HS, NH = 2, 2048, 1024, 64, 16
NCORE = 8
NTOK = B * T            # 4096
CH = 512                # token chunk
NCH = NTOK // CH        # 8
CPB = NCH // B          # chunks per batch = 4
SUB = 128
NSUB = CH // SUB        # 4

_nc_cache = {}


def build_nc():
    nc = bacc.Bacc("TRN2", target_bir_lowering=False, debug=False,
                   num_devices=NCORE)
    f32, bf16 = dt.float32, dt.bfloat16

    xP = nc.declare_dram_parameter("xP", [128, NCH, 8, CH], bf16,
                                   isOutput=False)
    wqT = nc.declare_dram_parameter("wqT", [128, 8, 128], bf16,
                                    isOutput=False)
    wkT = nc.declare_dram_parameter("wkT", [128, 8, 128], bf16,
                                    isOutput=False)
    wvT = nc.declare_dram_parameter("wvT", [128, 8, 128], bf16,
                                    isOutput=False)
    woAll = nc.declare_dram_parameter("woAll", [128, 8, E], bf16,
                                      isOutput=False)
    bqs = nc.declare_dram_parameter("bqs", [128, 1], f32, isOutput=False)
    bks = nc.declare_dram_parameter("bks", [128, 1], f32, isOutput=False)
    bvs = nc.declare_dram_parameter("bvs", [128, 1], f32, isOutput=False)
    bo_b = nc.declare_dram_parameter("bo_b", [128, E], f32, isOutput=False)
    eye = nc.declare_dram_parameter("eye", [128, 128], bf16, isOutput=False)
    tri01 = nc.declare_dram_parameter("tri01", [128, 128], bf16,
                                      isOutput=False)
    ones_v = nc.declare_dram_parameter("ones_v", [128, NCH * NSUB], bf16,
                                       isOutput=False)
    y = nc.declare_dram_parameter("y", [CH, E], bf16, isOutput=True)

    with tile.TileContext(nc) as tc:
        from contextlib import ExitStack
        with ExitStack() as top:
            const = top.enter_context(tc.tile_pool(name="const", bufs=1))
            persist = top.enter_context(tc.tile_pool(name="persist", bufs=1))
            xtp_pool = top.enter_context(tc.tile_pool(name="xtp", bufs=2))
            vstage = top.enter_context(tc.tile_pool(name="vstage", bufs=2))
            ppool = top.enter_context(tc.tile_pool(name="ppool", bufs=4))
            apool = top.enter_context(tc.tile_pool(name="apool", bufs=2))
            bcpool = top.enter_context(tc.tile_pool(name="bcpool", bufs=2))
            recpool = top.enter_context(tc.tile_pool(name="recpool", bufs=2))
            ystage = top.enter_context(tc.tile_pool(name="ystage", bufs=2))
            ps_q = top.enter_context(
                tc.tile_pool(name="ps_q", bufs=2, space="PSUM"))
            ps_s = top.enter_context(
                tc.tile_pool(name="ps_s", bufs=2, space="PSUM"))
            ps_a = top.enter_context(
                tc.tile_pool(name="ps_a", bufs=2, space="PSUM"))
            dram = top.enter_context(
                tc.tile_pool(name="dram", bufs=1, space="DRAM"))

            # ---- eye first: unblocks the HAM warmup matmuls ---------------
            eye_sb = const.tile([128, 128], bf16, name="eye_sb")
            nc.sync.dma_start(eye_sb[:], eye[:])

            # ---- HAM warmup: PE busy during the initial DMA wait ----------
            wps = ps_q.tile([128, 128], f32, name="wps", tag="psq")
            for _ in range(46):
                nc.tensor.matmul(wps[:], eye_sb[:], eye_sb[:],
                                 start=True, stop=True)

            # ---- persistent weights + chunk-0 data ------------------------
            wq_sb = persist.tile([128, 8, 128], bf16, name="wq_sb")
            wk_sb = persist.tile([128, 8, 128], bf16, name="wk_sb")
            wv_sb = persist.tile([128, 8, 128], bf16, name="wv_sb")
            nc.sync.dma_start(wq_sb[:], wqT[:])
            xTt0 = xtp_pool.tile([128, 8, CH], bf16, name="xTt", tag="xTt")
            nc.sync.dma_start(xTt0[:], xP[:, 0])
            nc.sync.dma_start(wk_sb[:], wkT[:])
            nc.sync.dma_start(wv_sb[:], wvT[:])

            # ---- remaining constants --------------------------------------
            onesv_sb = const.tile([128, NCH * NSUB], bf16, name="onesv_sb")
            nc.sync.dma_start(onesv_sb[:], ones_v[:])
            bq_sb = const.tile([128, 1], f32, name="bq_sb")
            nc.sync.dma_start(bq_sb[:], bqs[:])
            bk_sb = const.tile([128, 1], f32, name="bk_sb")
            nc.sync.dma_start(bk_sb[:], bks[:])
            bv_sb = const.tile([128, 1], f32, name="bv_sb")
            nc.sync.dma_start(bv_sb[:], bvs[:])
            tri_sb = const.tile([128, 128], bf16, name="tri_sb")
            nc.sync.dma_start(tri_sb[:], tri01[:])
            bo_sb = const.tile([128, E], f32, name="bo_sb")
            nc.sync.dma_start(bo_sb[:], bo_b[:])

            # ---- persistent activations -----------------------------------
            kT = persist.tile([128, NCH, CH], bf16, name="kT")
            qT = persist.tile([128, NCH, CH], bf16, name="qT")
            # V rows per k-token group g; cols 0:64 = h0 feats, 64 = ones,
            # 65:129 = h1 feats, 129 = ones.  AV stationary h = [:, g,
            # 65h:65h+65]; the ones row makes the AV matmul emit softmax
            # denominators in PSUM row 64.
            vh = persist.tile([128, NCH * NSUB, 130], bf16, name="vh")
            nc.vector.tensor_copy(vh[:, :, 64], onesv_sb[:])
            nc.vector.tensor_copy(vh[:, :, 129], onesv_sb[:])

            wo_sb = persist.tile([128, 8, E], bf16, name="wo_sb")

            # single A2A buffer: both head-halves stacked on the feat dim.
            # core c sends chunk j (its 128 feats x 512 tokens) to core j.
            cc_in = dram.tile([NCH, 128, CH], bf16, name="cc_in")
            cc_out = dram.tile([NCH, 128, CH], bf16, name="cc_out")
            warm_in = dram.tile([NCH, 1, 32], bf16, name="warm_in")
            warm_out = dram.tile([NCH, 1, 32], bf16, name="warm_out")
            nc.sync.dma_start(warm_in[:, 0, :], onesv_sb[0:8, 0:32])

            # ---- phase B: QKV projection for one token chunk ---------------
            def emit_b(t):
                if t == 0:
                    xTt = xTt0
                else:
                    xTt = xtp_pool.tile([128, 8, CH], bf16, name="xTt",
                                        tag="xTt")
                    nc.sync.dma_start(xTt[:], xP[:, t])
                for wsb, bias, scale, dest in (
                        (wq_sb, bq_sb, 0.125, qT),
                        (wk_sb, bk_sb, None, kT)):
                    ps = ps_q.tile([128, CH], f32, name="psqk", tag="psq")
                    for e in range(8):
                        nc.tensor.matmul(ps[:], wsb[:, e, :], xTt[:, e, :],
                                         start=(e == 0), stop=(e == 7))
                    if scale is None:
                        nc.vector.tensor_scalar_add(dest[:, t, :], ps[:],
                                                    bias[:])
                    else:
                        nc.vector.tensor_scalar(
                            dest[:, t, :], ps[:], scale, bias[:],
                            ALU.mult, ALU.add)

                psv = ps_q.tile([128, CH], f32, name="psv", tag="psq")
                for e in range(8):
                    nc.tensor.matmul(psv[:], wv_sb[:, e, :], xTt[:, e, :],
                                     start=(e == 0), stop=(e == 7))
                vTs = vstage.tile([128, CH], bf16, name="vTs", tag="vTs")
                nc.vector.tensor_scalar_add(vTs[:], psv[:], bv_sb[:])
                for s in range(NSUB):
                    tv = ps_q.tile([128, 128], bf16, name="tv", tag="psq")
                    nc.tensor.transpose(
                        tv[:], vTs[:, 128 * s:128 * (s + 1)], eye_sb[:])
                    g = NSUB * t + s
                    nc.vector.tensor_copy(vh[:, g, 0:64], tv[:, 0:64])
                    nc.vector.tensor_copy(vh[:, g, 65:129], tv[:, 64:128])

            # ---- phase C: attention for one (chunk, head-half) -------------
            def emit_c(t, h):
                b0 = CPB * (t // CPB)
                pb = 64 * h
                a_ps = ps_a.tile([128, CH], f32, name="a_ps", tag="aps")

                def emit_scores(kc):
                    diag = kc == t
                    pT = ppool.tile([128, NSUB, CH], bf16, name="pT",
                                    tag="pT")
                    for j in range(2):
                        sp = ps_s.tile([128, 2 * CH], f32, name="sp",
                                       tag="sps")
                        for jj in range(2):
                            s = 2 * j + jj
                            q0 = SUB * s if diag else 0
                            nc.tensor.matmul(
                                sp[:, CH * jj + q0:CH * jj + CH],
                                kT[pb:pb + 64, kc, SUB * s:SUB * (s + 1)],
                                qT[pb:pb + 64, t, q0:CH],
                                start=True, stop=True)
                        if diag:
                            for jj in range(2):
                                s = 2 * j + jj
                                q0 = SUB * s
                                nc.scalar.activation(
                                    pT[:, s, q0:CH],
                                    sp[:, CH * jj + q0:CH * jj + CH], AF.Exp)
                                nc.vector.tensor_mul(
                                    pT[:, s, q0:q0 + SUB],
                                    pT[:, s, q0:q0 + SUB], tri_sb[:])
                        else:
                            nc.scalar.activation(
                                pT[:, 2 * j:2 * j + 2, :], sp[:], AF.Exp)
                    return pT

                def emit_av(kc, pT):
                    diag = kc == t
                    for s in range(NSUB):
                        q0 = SUB * s if diag else 0
                        g = NSUB * kc + s
                        nc.tensor.matmul(
                            a_ps[0:65, q0:CH], vh[:, g, 65 * h:65 * h + 65],
                            pT[:, s, q0:CH],
                            start=(kc == b0 and s == 0),
                            stop=(diag and s == NSUB - 1))

                prev = None
                for kc in range(b0, t + 1):
                    pT = emit_scores(kc)
                    if prev is not None:
                        emit_av(*prev)
                    prev = (kc, pT)
                emit_av(*prev)

                # NOTE: the copy is load-bearing — it relocates PSUM row 64
                # to partition 0 (DVE custom ops are lane-aligned and cannot
                # cross partitions; reading a_ps[64:65] directly produces
                # garbage, verified on HW).
                den = recpool.tile([1, CH], f32, name="den", tag="den")
                nc.vector.tensor_copy(den[:], a_ps[64:65, :])
                rec = recpool.tile([1, CH], f32, name="rec", tag="rec")
                nc.vector.reciprocal_approx_fast(out=rec[:], in_=den[:])
                bc = bcpool.tile([64, CH], f32, name="bc", tag="bc")
                nc.gpsimd.partition_broadcast(bc[:], rec[:])
                a_sb = apool.tile([64, CH], bf16, name="a_sb", tag="asb")
                nc.vector.tensor_mul(a_sb[:], a_ps[0:64, :], bc[:])
                nc.sync.dma_start(cc_in[t, 64 * h:64 * h + 64, :], a_sb[:])

            # ---- main pipeline: QKV(t) | h0-attn(t-1) | h1-attn(t-2) ------
            for t in range(NCH):
                emit_b(t)
                if t == NCH - 1:
                    # tiny dummy AllToAll. Three roles: (1) keeps the CC
                    # stream warm for the real A2A; (2) re-syncs the cores
                    # here so accumulated thermal-throttle skew is absorbed
                    # while ~60us of attention work remains; (3) placed THIS
                    # late because collective_compute blocks the GpSimd
                    # queue until the collective completes, and the CC
                    # stream's init op can run until ~70us — an earlier
                    # dummy can queue behind it and stall the normalize
                    # chain (25-38us PE gaps observed at t==4/5/6).
                    nc.gpsimd.collective_compute(
                        "AllToAll", ALU.bypass,
                        ins=[warm_in.opt()], outs=[warm_out.opt()],
                        replica_groups=[list(range(NCORE))])
                if t >= 1:
                    emit_c(t - 1, 0)
                if t >= 2:
                    emit_c(t - 2, 1)
            # wo weights: DMA-idle window once all xP chunks are in flight
            nc.sync.dma_start(wo_sb[:], woAll[:])

            emit_c(NCH - 1, 0)
            emit_c(NCH - 2, 1)
            emit_c(NCH - 1, 1)
            # single A2A: both heads, all chunks (collectives here are
            # latency/skew-dominated, so one 1MB A2A beats two serialized
            # 512KB ones on the single CC stream)
            nc.gpsimd.collective_compute(
                "AllToAll", ALU.bypass,
                ins=[cc_in.opt()], outs=[cc_out.opt()],
                replica_groups=[list(range(NCORE))])

            # keep-warm: the A2A wait is 10-90us of PE idle; without this
            # the HAM re-throttles to K=4/8 and the whole output projection
            # runs at half clock (~9us loss, 634ns/mm observed). ~11us of
            # tiny matmuls keeps the array busy through the typical wait;
            # they overlap the collective and only delay the projection
            # when the A2A returns faster than that.
            wps2 = ps_q.tile([128, 64], f32, name="wps2", tag="psq")
            for _ in range(180):
                nc.tensor.matmul(wps2[:], eye_sb[:], eye_sb[:, 0:64],
                                 start=True, stop=True)

            # aTb[f, j, n]: feats of source core j for our 512 tokens.
            # Two tiles so the first projection chain (j=0..3) starts after
            # half the re-shard fetch instead of the full 1MB.
            aTbA = xtp_pool.tile([128, 4, CH], bf16, name="aTbA", tag="aTbA")
            aTbB = xtp_pool.tile([128, 4, CH], bf16, name="aTbB", tag="aTbB")
            nc.sync.dma_start(
                aTbA[:], cc_out[0:4].rearrange("j f n -> f j n"))
            nc.sync.dma_start(
                aTbB[:], cc_out[4:8].rearrange("j f n -> f j n"))

            # ---- output projection + store --------------------------------
            for m in range(NSUB):
                for nchk in range(2):
                    yps = ps_q.tile([128, CH], f32, name="yps", tag="psq")
                    for j in range(8):
                        src = aTbA if j < 4 else aTbB
                        nc.tensor.matmul(
                            yps[:],
                            src[:, j % 4, SUB * m:SUB * (m + 1)],
                            wo_sb[:, j, CH * nchk:CH * (nchk + 1)],
                            start=(j == 0), stop=(j == 7))
                    ysb = ystage.tile([128, CH], bf16, name="ysb", tag="ysb")
                    nc.vector.tensor_add(
                        ysb[:], yps[:],
                        bo_sb[:, CH * nchk:CH * (nchk + 1)])
                    nc.sync.dma_start(
                        y[SUB * m:SUB * (m + 1),
                          CH * nchk:CH * (nchk + 1)],
                        ysb[:])
    nc.compile()
    return nc


def _prep_in_maps(embd_q, Wq, bq, Wk, bk, Wv, bv, Wo, bo):
    import ml_dtypes
    bf16 = ml_dtypes.bfloat16
    x = embd_q.reshape(NTOK, E).astype(np.float32)
    # xP[p, t, e, n] = x[t*512+n, e*128+p]: 8KB contiguous per partition
    # per chunk
    xPm = np.ascontiguousarray(
        x.reshape(NCH, CH, 8, 128).transpose(3, 0, 2, 1).astype(bf16))
    eye = np.eye(128, dtype=bf16)
    r = np.arange(128)
    # pT is [k-part, q-col]; mask out k > q (future tokens)
    tri01 = np.ascontiguousarray(
        np.where(r[:, None] > r[None, :], 0.0, 1.0).astype(bf16))
    ones_v = np.ones((128, NCH * NSUB), dtype=bf16)
    bo_b = np.ascontiguousarray(
        np.broadcast_to(bo.astype(np.float32), (128, E)))
    woTf = Wo.astype(np.float32).T  # [feat, out]
    # woAll[p, j, o] = Wo.T[128j + p, o]: cc_out[j] row p is global feat
    # 128j + p, so the projection contracts per-source-core tiles
    woAllm = np.ascontiguousarray(
        woTf.reshape(8, 128, E).transpose(1, 0, 2).astype(bf16))

    def wlayout(W, sl):
        # [E, 128] -> [p, e, m]: contiguous 2KB/partition DMA segments
        wT = W[sl].astype(np.float32).T.astype(bf16)
        return np.ascontiguousarray(
            wT.reshape(8, 128, 128).transpose(1, 0, 2))

    in_maps = []
    for c in range(NCORE):
        sl = slice(128 * c, 128 * (c + 1))
        in_maps.append({
            "xP": xPm,
            "wqT": wlayout(Wq, sl),
            "wkT": wlayout(Wk, sl),
            "wvT": wlayout(Wv, sl),
            "woAll": woAllm,
            "bqs": np.ascontiguousarray(
                (bq[sl] * 0.125).reshape(128, 1), dtype=np.float32),
            "bks": np.ascontiguousarray(bk[sl].reshape(128, 1),
                                        dtype=np.float32),
            "bvs": np.ascontiguousarray(bv[sl].reshape(128, 1),
                                        dtype=np.float32),
            "bo_b": bo_b,
            "eye": eye,
            "tri01": tri01,
            "ones_v": ones_v,
        })
    return in_maps


def kernel(embd_q, Wq, bq, Wk, bk, Wv, bv, Wo, bo, _trace=False):
    if "nc" not in _nc_cache:
        _nc_cache["nc"] = build_nc()
    in_maps = _prep_in_maps(np.asarray(embd_q), np.asarray(Wq), np.asarray(bq),
                            np.asarray(Wk), np.asarray(bk), np.asarray(Wv),
                            np.asarray(bv), np.asarray(Wo), np.asarray(bo))
    import os
    tc_env = os.environ.get("TRACE_CORES")
    res = run_bass_kernel_spmd(
        _nc_cache["nc"], in_maps, list(range(NCORE)), trace=_trace,
        trace_cores=(list(range(NCORE)) if tc_env else None))
    out = np.concatenate(
        [np.asarray(res.results[c]["y"]).astype(np.float32)
         for c in range(NCORE)], axis=0)
    out = out.reshape(B, T, E)
    kernel.last_results = res
    return out
